# revision 1
# baseline (speedup 1.0000x reference)
"""Trainium2 Bass kernel for nn_Conv_6511170421767.

3x3 conv, stride 1, pad 1 on x:(32,128,56,56) with weight:(256,128,3,3),
bias:(256,) -> out:(32,256,56,56), fp32 in/out.

Strategy (data-parallel, 4 images per core on 8 cores):
- Cin=128 is exactly the PE contraction/partition dim. The conv becomes,
  per (output-row-block, Cout-chunk), an accumulation of 9 matmuls (one per
  kernel tap): out[co, pix] += W[dr,dc][ci,co].T @ xpad[ci, shifted pix].
- The host pre-pads x to (4,128,58,58) fp16 and pre-transposes/casts the
  weights, so input DMAs land directly in the padded SBUF plane and the
  on-chip DVE does ZERO work (v1 spent ~26us of DVE on casts/borders).
- Matmul operands are fp16 (1 PE cycle/row - vs 4 for plain fp32 - with a
  10-bit mantissa; operand ranges here sit safely inside fp16's range).
  Accumulation is fp32 in PSUM. Measured rel err vs fp32 reference: ~3e-4.
- PSUM tile [128, 448] = one bank; 9 taps accumulate in-bank, then the
  scalar engine adds bias (Identity activation w/ per-partition bias AP)
  while copying PSUM->SBUF as fp16; the host upcasts fp16->fp32. The very
  last tile is split into row-halves to shorten the final ACT+DMA+
  completion chain. Output DMAs ride the sync ring (it keeps pace; ~0.6us
  descriptor cost each, paced ~1.7us apart by compute).
- Cold-start: the PE HAM clock gate runs matmuls at 1.2GHz until ~3.4us of
  CONTINUOUS PE-busy (any idle gap restarts the window). Dependency-free
  warmup matmuls (zero fp16 operands from a DVE memset, never read back)
  start right after the ~7.5us fixed framework preamble and bridge into
  the first data-gated matmul, so the HAM flips to 2.4GHz ~1.8us into the
  real stream. Startup transfers are deadline-ordered across BOTH HWDGE
  rings: sync = [x rows 0-9, taps 5-8, x rows 10-25, bias, x rest, img1,
  img2], scalar = [taps 0-1, taps 2-4] (kept free of bulk so the tap
  completion semaphores land before their first LDWEIGHTS; a transfer's
  completion lags its issue by ~2-2.5us). Each DMA's completion semaphore
  is per-transfer, hence the deadline-sized pieces.

The external neuronxcc walrus in this container enforces small per-
instruction sync-wait limits (TRN2 HW allows 1 per instruction);
_cap_sync_waits() splits excess waits onto InstNoOp instructions inserted
just before the offender on the same engine.
"""

import sys

sys.path.insert(0, "/opt/trn_rl_repo")

import numpy as np

import concourse.bass as bass
import concourse.mybir as mybir
import concourse.tile as tile
from concourse.bass_utils import run_bass_kernel_spmd

F32 = mybir.dt.float32
FP16 = mybir.dt.float16

N_CORES = 8
IMGS_PER_CORE = 4
CIN = 128
COUT = 256
H = W = 56
HP = WP = 58  # padded plane
ROWS_PER_TILE = 8  # 8 output rows -> N = 448 <= 512 (one PSUM bank)
N_ROW_TILES = H // ROWS_PER_TILE  # 7
NTILE = ROWS_PER_TILE * W  # 448
N_WARM = 8  # dependency-free HAM-warmup matmuls
IMG0_CHUNKS = [0, 10, 26, 42, 58]  # row bands; tile t needs rows 8t..8t+10

# Per-instruction sync-wait budget for the external walrus: TRN2 hardware
# allows at most 1 sync wait per instruction.
_WAIT_LIMITS_DEFAULT = 1
_WAIT_LIMITS = {}


def _cap_sync_waits(nc):
    """Split sync waits exceeding per-instruction limits onto same-engine
    InstNoOp instructions inserted immediately before the offender."""
    for fn in nc.m.functions:
        for bb in fn.blocks:
            i = 0
            insts = bb.instructions
            while i < len(insts):
                inst = insts[i]
                si = getattr(inst, "sync_info", None)
                if si is None or not si.on_wait:
                    i += 1
                    continue
                limit = _WAIT_LIMITS.get(type(inst).__name__, _WAIT_LIMITS_DEFAULT)
                waits = list(si.on_wait)
                if len(waits) <= limit:
                    i += 1
                    continue
                keep = waits[:limit]
                excess = waits[limit:]
                inst.sync_info = mybir.SyncInfo(
                    on_wait=keep, on_update=list(si.on_update)
                )
                pos = i
                for j in range(0, len(excess), _WAIT_LIMITS_DEFAULT):
                    chunk = excess[j : j + _WAIT_LIMITS_DEFAULT]
                    nop = mybir.InstNoOp(
                        name=nc.get_next_instruction_name(), ins=[], outs=[]
                    )
                    nop.engine = inst.engine
                    nop.sync_info = mybir.SyncInfo(on_wait=chunk, on_update=[])
                    nc.register_instruction(nop)
                    insts.insert(pos, nop)
                    pos += 1
                    i += 1
                i += 1


def build_conv_nc():
    """One-core program: x:(4,128,58,58) fp16 (pre-padded), wT:(128,9*256)
    fp16, bias2:(128,2) f32 -> out:(4,256,56,56) fp16."""
    nc = bass.Bass()
    x = nc.dram_tensor("x", [IMGS_PER_CORE, CIN, HP, WP], FP16, kind="ExternalInput")
    wt = nc.dram_tensor("wT", [CIN, 9 * COUT], FP16, kind="ExternalInput")
    bias2 = nc.dram_tensor("bias2", [128, 2], F32, kind="ExternalInput")
    out = nc.dram_tensor(
        "out", [IMGS_PER_CORE, COUT, H, W], FP16, kind="ExternalOutput"
    )

    with tile.TileContext(nc) as tc:
        with (
            tc.tile_pool(name="const", bufs=1) as const_pool,
            tc.tile_pool(name="xpad", bufs=1) as xpad_pool,
            tc.tile_pool(name="obuf", bufs=4) as obuf_pool,
            tc.tile_pool(name="psum", bufs=8, space="PSUM") as psum_pool,
        ):
            w_sb = const_pool.tile([CIN, 9 * COUT], FP16)
            b_sb = const_pool.tile([128, 2], F32)
            wz = const_pool.tile([CIN, NTILE], FP16)
            xpads = [
                xpad_pool.tile([CIN, HP, WP], FP16, tag=f"xpad{i}", name=f"xpad{i}")
                for i in range(4)
            ]

            # HAM warmup: memset-only dependency, so these issue right after
            # the framework preamble and keep the PE busy while input DMAs
            # stream. Results are never read.
            nc.vector.memset(wz[:], 0.0)
            for i in range(N_WARM):
                pw = psum_pool.tile([128, NTILE], F32, tag="ps", name=f"warm{i}")
                nc.tensor.matmul(pw[:], wz[:, 0:128], wz[:], start=True, stop=True)

            # Startup DMAs run on BOTH HWDGE rings in parallel, ordered by
            # first-use deadline (tap k of tile0 is needed ~0.4us*k after
            # MM#1; each transfer's completion semaphore lags its data by
            # ~1us, so weights move in three deadline-sized pieces).
            # Layout wT[ci, (tap, chunk, co128)].
            nc.sync.dma_start(xpads[0][:, 0:10, :], x[0, :, 0:10, :])
            nc.sync.dma_start(w_sb[:, 10 * 128 :], wt[:, 10 * 128 :])
            nc.sync.dma_start(
                xpads[0][:, 10 : IMG0_CHUNKS[2], :], x[0, :, 10 : IMG0_CHUNKS[2], :]
            )
            nc.sync.dma_start(b_sb[:], bias2[:])
            nc.scalar.dma_start(w_sb[:, 0 : 4 * 128], wt[:, 0 : 4 * 128])
            nc.scalar.dma_start(w_sb[:, 4 * 128 : 10 * 128], wt[:, 4 * 128 : 10 * 128])
            # img1/2 prefetches are issued from INSIDE the loop: all logical
            # DMA queues share the 16 SDMA engines, so any bulk issued during
            # the startup window delays the tap-weight completion semaphores
            # past their LDWEIGHTS deadlines (measured 3us straggle) no
            # matter which ring carries it.

            for img in range(IMGS_PER_CORE):
                xp = xpads[img]
                for t in range(N_ROW_TILES):
                    if img == 0 and t == 1:
                        for ci in range(2, len(IMG0_CHUNKS) - 1):
                            r0c = IMG0_CHUNKS[ci]
                            r1c = IMG0_CHUNKS[ci + 1]
                            nc.sync.dma_start(
                                xpads[0][:, r0c:r1c, :], x[0, :, r0c:r1c, :]
                            )
                        nc.sync.dma_start(xpads[1][:], x[1])
                    if img == 0 and t == 5:
                        nc.sync.dma_start(xpads[2][:], x[2])
                    if img == 1 and t == 2:
                        nc.scalar.dma_start(xpads[3][:], x[3])
                    y0 = t * ROWS_PER_TILE
                    # The very last tile is split into row-halves so the
                    # final ACT+DMA+completion chain after the last matmul
                    # is half as deep.
                    last = img == IMGS_PER_CORE - 1 and t == N_ROW_TILES - 1
                    spans = (
                        [(y0, 4), (y0 + 4, 2), (y0 + 6, 2)]
                        if last
                        else [(y0, ROWS_PER_TILE)]
                    )
                    if img == 0 and t == 0:
                        # Consume taps 0/1 for BOTH chunks first: taps 2-4
                        # arrive on the scalar ring ~1us after taps 0-1, so
                        # this buys their completion an extra ~0.75us.
                        pss = [
                            psum_pool.tile(
                                [128, NTILE], F32, tag="ps", name=f"ps_0_0_{c}"
                            )
                            for c in range(2)
                        ]
                        t0_order = [(0, 0), (0, 1), (1, 0), (1, 1)] + [
                            (k, c) for c in range(2) for k in range(2, 9)
                        ]
                        for k, c in t0_order:
                            dr, dc = divmod(k, 3)
                            nc.tensor.matmul(
                                pss[c][:],
                                w_sb[:, (k * 2 + c) * 128 : (k * 2 + c) * 128 + 128],
                                xp[:, dr : dr + ROWS_PER_TILE, dc : dc + W],
                                start=(k == 0),
                                stop=(k == 8),
                            )
                        for c in range(2):
                            ob = obuf_pool.tile(
                                [128, ROWS_PER_TILE, W], FP16, tag="ob",
                                name=f"ob_0_0_{c}",
                            )
                            nc.scalar.activation(
                                ob[:],
                                pss[c][:].rearrange("p (r w) -> p r w", w=W),
                                mybir.ActivationFunctionType.Identity,
                                bias=b_sb[:, c : c + 1],
                                scale=1.0,
                            )
                            nc.sync.dma_start(
                                out[0, c * 128 : (c + 1) * 128, 0:ROWS_PER_TILE, :],
                                ob[:],
                            )
                        continue
                    for c in range(2):  # Cout chunks of 128
                        for r0, nr in spans:
                            nt = nr * W
                            ps = psum_pool.tile(
                                [128, nt], F32, tag="ps", name=f"ps_{img}_{r0}_{c}"
                            )
                            for k in range(9):
                                dr, dc = divmod(k, 3)
                                lhsT = w_sb[
                                    :, (k * 2 + c) * 128 : (k * 2 + c) * 128 + 128
                                ]
                                rhs = xp[:, r0 + dr : r0 + dr + nr, dc : dc + W]
                                nc.tensor.matmul(
                                    ps[:], lhsT, rhs, start=(k == 0), stop=(k == 8)
                                )
                            ob = obuf_pool.tile(
                                [128, nr, W], FP16, tag="ob",
                                name=f"ob_{img}_{r0}_{c}",
                            )
                            # out = Identity(psum * 1.0 + bias[co]) on ScalarE
                            nc.scalar.activation(
                                ob[:],
                                ps[:].rearrange("p (r w) -> p r w", w=W),
                                mybir.ActivationFunctionType.Identity,
                                bias=b_sb[:, c : c + 1],
                                scale=1.0,
                            )
                            nc.sync.dma_start(
                                out[img, c * 128 : (c + 1) * 128, r0 : r0 + nr, :],
                                ob[:],
                            )

    _cap_sync_waits(nc)
    nc.finalize()
    return nc


_NC_CACHE = {}


def _get_nc():
    if "nc" not in _NC_CACHE:
        _NC_CACHE["nc"] = build_conv_nc()
    return _NC_CACHE["nc"]


def _prep_in_maps(x, weight, bias):
    x = np.asarray(x, dtype=np.float32)
    n = x.shape[0]
    # pad to 58x58 and cast fp16 once, full batch
    xp = np.zeros((n, CIN, HP, WP), dtype=np.float16)
    xp[:, :, 1 : H + 1, 1 : W + 1] = x
    # weight (256,128,3,3) -> wT[ci, (tap, chunk, co128)] fp16
    wT = (
        np.transpose(np.asarray(weight, dtype=np.float32), (1, 2, 3, 0))
        .reshape(CIN, 9, 2, 128)
        .reshape(CIN, 9 * COUT)
        .astype(np.float16)
    )
    wT = np.ascontiguousarray(wT)
    bias2 = np.ascontiguousarray(
        np.asarray(bias, dtype=np.float32).reshape(2, 128).T
    )
    per_core = n // N_CORES
    return [
        {
            "x": np.ascontiguousarray(xp[i * per_core : (i + 1) * per_core]),
            "wT": wT,
            "bias2": bias2,
        }
        for i in range(N_CORES)
    ]


def run(x, weight, bias, trace=False):
    """Run the conv on 8 cores; returns (out, BassKernelResults)."""
    nc = _get_nc()
    in_maps = _prep_in_maps(x, weight, bias)
    res = run_bass_kernel_spmd(
        nc, in_maps, core_ids=list(range(N_CORES)), trace=trace
    )
    out = np.concatenate([r["out"] for r in res.results], axis=0).astype(np.float32)
    return out, res


def kernel(x, weight, bias):
    out, _ = run(x, weight, bias, trace=False)
    return out



# revision 5
# speedup vs baseline: 1.0063x; 1.0063x over previous
"""Trainium2 Bass kernel for nn_Conv_6511170421767.

3x3 conv, stride 1, pad 1 on x:(32,128,56,56) with weight:(256,128,3,3),
bias:(256,) -> out:(32,256,56,56), fp32 in/out.

Strategy (data-parallel, 4 images per core on 8 cores):
- Cin=128 is exactly the PE contraction/partition dim. The conv becomes,
  per (output-row-block, Cout-chunk), an accumulation of 9 matmuls (one per
  kernel tap): out[co, pix] += W[dr,dc][ci,co].T @ xpad[ci, shifted pix].
- The host pre-pads x to (4,128,58,58) fp16 and pre-transposes/casts the
  weights, so input DMAs land directly in the padded SBUF plane and the
  on-chip DVE does ZERO work (v1 spent ~26us of DVE on casts/borders).
- Matmul operands are fp16 (1 PE cycle/row - vs 4 for plain fp32 - with a
  10-bit mantissa; operand ranges here sit safely inside fp16's range).
  Accumulation is fp32 in PSUM. Measured rel err vs fp32 reference: ~3e-4.
- PSUM tile [128, 448] = one bank; 9 taps accumulate in-bank, then the
  scalar engine adds bias (Identity activation w/ per-partition bias AP)
  while copying PSUM->SBUF as fp16; the host upcasts fp16->fp32. The very
  last tile is split into row-halves to shorten the final ACT+DMA+
  completion chain. Output DMAs ride the sync ring (it keeps pace; ~0.6us
  descriptor cost each, paced ~1.7us apart by compute).
- Cold-start: the PE HAM clock gate runs matmuls at 1.2GHz until ~3.4us of
  CONTINUOUS PE-busy (any idle gap restarts the window). Dependency-free
  warmup matmuls (zero fp16 operands from a DVE memset, never read back)
  start right after the ~7.5us fixed framework preamble and bridge into
  the first data-gated matmul, so the HAM flips to 2.4GHz ~1.8us into the
  real stream. Startup transfers are deadline-ordered across BOTH HWDGE
  rings: sync = [x rows 0-9, taps 5-8, x rows 10-25, bias, x rest, img1,
  img2], scalar = [taps 0-1, taps 2-4] (kept free of bulk so the tap
  completion semaphores land before their first LDWEIGHTS; a transfer's
  completion lags its issue by ~2-2.5us). Each DMA's completion semaphore
  is per-transfer, hence the deadline-sized pieces.

The external neuronxcc walrus in this container enforces small per-
instruction sync-wait limits (TRN2 HW allows 1 per instruction);
_cap_sync_waits() splits excess waits onto InstNoOp instructions inserted
just before the offender on the same engine.
"""

import sys

sys.path.insert(0, "/opt/trn_rl_repo")

import numpy as np

import concourse.bass as bass
import concourse.mybir as mybir
import concourse.tile as tile
from concourse.bass_utils import run_bass_kernel_spmd

F32 = mybir.dt.float32
FP16 = mybir.dt.float16

N_CORES = 8
IMGS_PER_CORE = 4
CIN = 128
COUT = 256
H = W = 56
HP = WP = 58  # padded plane
ROWS_PER_TILE = 8  # 8 output rows -> N = 448 <= 512 (one PSUM bank)
N_ROW_TILES = H // ROWS_PER_TILE  # 7
NTILE = ROWS_PER_TILE * W  # 448
N_WARM = 15  # dependency-free HAM-warmup matmuls (N=224 each, ~190ns cold)
NWZ = 224  # warmup matmul free dim

# Per-instruction sync-wait budget for the external walrus: TRN2 hardware
# allows at most 1 sync wait per instruction.
_WAIT_LIMITS_DEFAULT = 1
_WAIT_LIMITS = {}


def _cap_sync_waits(nc):
    """Split sync waits exceeding per-instruction limits onto same-engine
    InstNoOp instructions inserted immediately before the offender."""
    for fn in nc.m.functions:
        for bb in fn.blocks:
            i = 0
            insts = bb.instructions
            while i < len(insts):
                inst = insts[i]
                si = getattr(inst, "sync_info", None)
                if si is None or not si.on_wait:
                    i += 1
                    continue
                limit = _WAIT_LIMITS.get(type(inst).__name__, _WAIT_LIMITS_DEFAULT)
                waits = list(si.on_wait)
                if len(waits) <= limit:
                    i += 1
                    continue
                keep = waits[:limit]
                excess = waits[limit:]
                inst.sync_info = mybir.SyncInfo(
                    on_wait=keep, on_update=list(si.on_update)
                )
                pos = i
                for j in range(0, len(excess), _WAIT_LIMITS_DEFAULT):
                    chunk = excess[j : j + _WAIT_LIMITS_DEFAULT]
                    nop = mybir.InstNoOp(
                        name=nc.get_next_instruction_name(), ins=[], outs=[]
                    )
                    nop.engine = inst.engine
                    nop.sync_info = mybir.SyncInfo(on_wait=chunk, on_update=[])
                    nc.register_instruction(nop)
                    insts.insert(pos, nop)
                    pos += 1
                    i += 1
                i += 1


def build_conv_nc():
    """One-core program: x:(4,128,58,58) fp16 (pre-padded), wT:(128,9*256)
    fp16, bias2:(128,2) f32 -> out:(4,256,56,56) fp16."""
    nc = bass.Bass()
    x = nc.dram_tensor("x", [IMGS_PER_CORE, CIN, HP, WP], FP16, kind="ExternalInput")
    wt = nc.dram_tensor("wT", [CIN, 9 * COUT], FP16, kind="ExternalInput")
    bias2 = nc.dram_tensor("bias2", [128, 2], F32, kind="ExternalInput")
    out = nc.dram_tensor(
        "out", [IMGS_PER_CORE, COUT, H, W], FP16, kind="ExternalOutput"
    )

    with tile.TileContext(nc) as tc:
        with (
            tc.tile_pool(name="const", bufs=1) as const_pool,
            tc.tile_pool(name="xpad", bufs=1) as xpad_pool,
            tc.tile_pool(name="obuf", bufs=4) as obuf_pool,
            tc.tile_pool(name="psum", bufs=8, space="PSUM") as psum_pool,
        ):
            w_sb = const_pool.tile([CIN, 9 * COUT], FP16)
            b_sb = const_pool.tile([128, 2], F32)
            wz = const_pool.tile([CIN, NWZ], FP16)
            xpads = [
                xpad_pool.tile([CIN, HP, WP], FP16, tag=f"xpad{i}", name=f"xpad{i}")
                for i in range(4)
            ]

            # HAM warmup: memset-only dependency, so these issue right after
            # the framework preamble and keep the PE busy while input DMAs
            # stream. Results are never read.
            nc.vector.memset(wz[:], 0.0)
            for i in range(N_WARM):
                pw = psum_pool.tile([128, NWZ], F32, tag="ps", name=f"warm{i}")
                nc.tensor.matmul(pw[:], wz[:, 0:128], wz[:], start=True, stop=True)

            # Startup DMAs: rings split by ROLE. Each dma_start costs
            # ~0.6-1.2us of serial descriptor-generation (DIRECT2D) on its
            # ring's sequencer, and all rings share the 16 SDMA engines
            # round-robin. Weights (582KB, deadline ~11-15us) ride the
            # scalar/ACT ring alone so their completion semaphores are not
            # queued behind image bulk; x planes (3.4MB total) stream on the
            # sync ring in deadline order. bias + img3 trail on scalar.
            # Layout wT[ci, (tap, chunk, co128)].
            nc.sync.dma_start(xpads[0][:, 0:10, :], x[0, :, 0:10, :])
            nc.sync.dma_start(xpads[0][:, 10:26, :], x[0, :, 10:26, :])
            nc.sync.dma_start(xpads[0][:, 26:42, :], x[0, :, 26:42, :])
            nc.sync.dma_start(xpads[0][:, 42:58, :], x[0, :, 42:58, :])
            nc.sync.dma_start(xpads[1][:], x[1])
            nc.sync.dma_start(xpads[2][:], x[2])
            nc.scalar.dma_start(w_sb[:, 0 : 4 * 128], wt[:, 0 : 4 * 128])
            nc.scalar.dma_start(w_sb[:, 4 * 128 : 10 * 128], wt[:, 4 * 128 : 10 * 128])
            nc.scalar.dma_start(w_sb[:, 10 * 128 :], wt[:, 10 * 128 :])
            nc.scalar.dma_start(b_sb[:], bias2[:])
            nc.scalar.dma_start(xpads[3][:], x[3])

            for img in range(IMGS_PER_CORE):
                xp = xpads[img]
                for t in range(N_ROW_TILES):
                    y0 = t * ROWS_PER_TILE
                    # The very last tile is split into row-halves so the
                    # final ACT+DMA+completion chain after the last matmul
                    # is half as deep.
                    last = img == IMGS_PER_CORE - 1 and t == N_ROW_TILES - 1
                    spans = (
                        [(y0, 4), (y0 + 4, 2), (y0 + 6, 2)]
                        if last
                        else [(y0, ROWS_PER_TILE)]
                    )
                    if img == 0 and t == 0:
                        # Consume taps 0/1 for BOTH chunks first: taps 2-4
                        # arrive on the scalar ring ~1us after taps 0-1, so
                        # this buys their completion an extra ~0.75us.
                        pss = [
                            psum_pool.tile(
                                [128, NTILE], F32, tag="ps", name=f"ps_0_0_{c}"
                            )
                            for c in range(2)
                        ]
                        t0_order = [(0, 0), (0, 1), (1, 0), (1, 1)] + [
                            (k, c) for c in range(2) for k in range(2, 9)
                        ]
                        for k, c in t0_order:
                            dr, dc = divmod(k, 3)
                            nc.tensor.matmul(
                                pss[c][:],
                                w_sb[:, (k * 2 + c) * 128 : (k * 2 + c) * 128 + 128],
                                xp[:, dr : dr + ROWS_PER_TILE, dc : dc + W],
                                start=(k == 0),
                                stop=(k == 8),
                            )
                        for c in range(2):
                            ob = obuf_pool.tile(
                                [128, ROWS_PER_TILE, W], FP16, tag="ob",
                                name=f"ob_0_0_{c}",
                            )
                            nc.scalar.activation(
                                ob[:],
                                pss[c][:].rearrange("p (r w) -> p r w", w=W),
                                mybir.ActivationFunctionType.Identity,
                                bias=b_sb[:, c : c + 1],
                                scale=1.0,
                            )
                            oring = nc.sync if c == 0 else nc.scalar
                            oring.dma_start(
                                out[0, c * 128 : (c + 1) * 128, 0:ROWS_PER_TILE, :],
                                ob[:],
                            )
                        continue
                    for c in range(2):  # Cout chunks of 128
                        for si, (r0, nr) in enumerate(spans):
                            nt = nr * W
                            ps = psum_pool.tile(
                                [128, nt], F32, tag="ps", name=f"ps_{img}_{r0}_{c}"
                            )
                            for k in range(9):
                                dr, dc = divmod(k, 3)
                                lhsT = w_sb[
                                    :, (k * 2 + c) * 128 : (k * 2 + c) * 128 + 128
                                ]
                                rhs = xp[:, r0 + dr : r0 + dr + nr, dc : dc + W]
                                nc.tensor.matmul(
                                    ps[:], lhsT, rhs, start=(k == 0), stop=(k == 8)
                                )
                            ob = obuf_pool.tile(
                                [128, nr, W], FP16, tag="ob",
                                name=f"ob_{img}_{r0}_{c}",
                            )
                            # out = Identity(psum * 1.0 + bias[co]) on ScalarE
                            nc.scalar.activation(
                                ob[:],
                                ps[:].rearrange("p (r w) -> p r w", w=W),
                                mybir.ActivationFunctionType.Identity,
                                bias=b_sb[:, c : c + 1],
                                scale=1.0,
                            )
                            # c0 -> sync ring, c1 -> scalar ring; within the
                            # split last tile, interleave spans across rings
                            # so consecutive descriptor generations never
                            # serialize on one ring at the very end.
                            oring = nc.sync if (c + si) % 2 == 0 else nc.scalar
                            oring.dma_start(
                                out[img, c * 128 : (c + 1) * 128, r0 : r0 + nr, :],
                                ob[:],
                            )

    _cap_sync_waits(nc)
    nc.finalize()
    return nc


_NC_CACHE = {}


def _get_nc():
    if "nc" not in _NC_CACHE:
        _NC_CACHE["nc"] = build_conv_nc()
    return _NC_CACHE["nc"]


def _prep_in_maps(x, weight, bias):
    x = np.asarray(x, dtype=np.float32)
    n = x.shape[0]
    # pad to 58x58 and cast fp16 once, full batch
    xp = np.zeros((n, CIN, HP, WP), dtype=np.float16)
    xp[:, :, 1 : H + 1, 1 : W + 1] = x
    # weight (256,128,3,3) -> wT[ci, (tap, chunk, co128)] fp16
    wT = (
        np.transpose(np.asarray(weight, dtype=np.float32), (1, 2, 3, 0))
        .reshape(CIN, 9, 2, 128)
        .reshape(CIN, 9 * COUT)
        .astype(np.float16)
    )
    wT = np.ascontiguousarray(wT)
    bias2 = np.ascontiguousarray(
        np.asarray(bias, dtype=np.float32).reshape(2, 128).T
    )
    per_core = n // N_CORES
    return [
        {
            "x": np.ascontiguousarray(xp[i * per_core : (i + 1) * per_core]),
            "wT": wT,
            "bias2": bias2,
        }
        for i in range(N_CORES)
    ]


def run(x, weight, bias, trace=False):
    """Run the conv on 8 cores; returns (out, BassKernelResults)."""
    nc = _get_nc()
    in_maps = _prep_in_maps(x, weight, bias)
    res = run_bass_kernel_spmd(
        nc, in_maps, core_ids=list(range(N_CORES)), trace=trace
    )
    out = np.concatenate([r["out"] for r in res.results], axis=0).astype(np.float32)
    return out, res


def kernel(x, weight, bias):
    out, _ = run(x, weight, bias, trace=False)
    return out



# revision 10
# speedup vs baseline: 1.0154x; 1.0091x over previous
"""Trainium2 Bass kernel for nn_Conv_6511170421767.

3x3 conv, stride 1, pad 1 on x:(32,128,56,56) with weight:(256,128,3,3),
bias:(256,) -> out:(32,256,56,56), fp32 in/out.

Strategy (data-parallel, 4 images per core on 8 cores):
- Cin=128 is exactly the PE contraction/partition dim. The conv becomes,
  per (output-row-block, Cout-chunk), an accumulation of 9 matmuls (one per
  kernel tap): out[co, pix] += W[dr,dc][ci,co].T @ xpad[ci, shifted pix].
- The host pre-pads x to (4,128,58,58) fp16 and pre-transposes/casts the
  weights, so input DMAs land directly in the padded SBUF plane and the
  on-chip DVE does ZERO work (v1 spent ~26us of DVE on casts/borders).
- Matmul operands are fp16 (1 PE cycle/row - vs 4 for plain fp32 - with a
  10-bit mantissa; operand ranges here sit safely inside fp16's range).
  Accumulation is fp32 in PSUM. Measured rel err vs fp32 reference: ~3e-4.
- PSUM tile [128, 448] = one bank; 9 taps accumulate in-bank, then the
  scalar engine adds bias (Identity activation w/ per-partition bias AP)
  while copying PSUM->SBUF as fp16; the host upcasts fp16->fp32. The very
  last tile is split into row-halves to shorten the final ACT+DMA+
  completion chain. Output DMAs ride the sync ring (it keeps pace; ~0.6us
  descriptor cost each, paced ~1.7us apart by compute).
- Cold-start: the PE HAM clock gate runs matmuls at 1.2GHz until ~3.4us of
  CONTINUOUS PE-busy (any idle gap restarts the window). Dependency-free
  warmup matmuls (zero fp16 operands from a DVE memset, never read back)
  start right after the ~7.5us fixed framework preamble and bridge into
  the first data-gated matmul, so the HAM flips to 2.4GHz ~1.8us into the
  real stream. Startup transfers are deadline-ordered across BOTH HWDGE
  rings: sync = [x rows 0-9, taps 5-8, x rows 10-25, bias, x rest, img1,
  img2], scalar = [taps 0-1, taps 2-4] (kept free of bulk so the tap
  completion semaphores land before their first LDWEIGHTS; a transfer's
  completion lags its issue by ~2-2.5us). Each DMA's completion semaphore
  is per-transfer, hence the deadline-sized pieces.

The external neuronxcc walrus in this container enforces small per-
instruction sync-wait limits (TRN2 HW allows 1 per instruction);
_cap_sync_waits() splits excess waits onto InstNoOp instructions inserted
just before the offender on the same engine.
"""

import sys

sys.path.insert(0, "/opt/trn_rl_repo")

import numpy as np

import concourse.bass as bass
import concourse.mybir as mybir
import concourse.tile as tile
from concourse.bass_utils import run_bass_kernel_spmd

F32 = mybir.dt.float32
FP16 = mybir.dt.float16

N_CORES = 8
IMGS_PER_CORE = 4
CIN = 128
COUT = 256
H = W = 56
HP = WP = 58  # padded plane
ROWS_PER_TILE = 8  # 8 output rows -> N = 448 <= 512 (one PSUM bank)
N_ROW_TILES = H // ROWS_PER_TILE  # 7
NTILE = ROWS_PER_TILE * W  # 448
N_WARM = 12  # dependency-free HAM-warmup matmuls (N=224 each, ~190ns cold)
NWZ = 224  # warmup matmul free dim

# Per-instruction sync-wait budget for the external walrus: TRN2 hardware
# allows at most 1 sync wait per instruction.
_WAIT_LIMITS_DEFAULT = 1
_WAIT_LIMITS = {}


def _cap_sync_waits(nc):
    """Split sync waits exceeding per-instruction limits onto same-engine
    InstNoOp instructions inserted immediately before the offender."""
    for fn in nc.m.functions:
        for bb in fn.blocks:
            i = 0
            insts = bb.instructions
            while i < len(insts):
                inst = insts[i]
                si = getattr(inst, "sync_info", None)
                if si is None or not si.on_wait:
                    i += 1
                    continue
                limit = _WAIT_LIMITS.get(type(inst).__name__, _WAIT_LIMITS_DEFAULT)
                waits = list(si.on_wait)
                if len(waits) <= limit:
                    i += 1
                    continue
                keep = waits[:limit]
                excess = waits[limit:]
                inst.sync_info = mybir.SyncInfo(
                    on_wait=keep, on_update=list(si.on_update)
                )
                pos = i
                for j in range(0, len(excess), _WAIT_LIMITS_DEFAULT):
                    chunk = excess[j : j + _WAIT_LIMITS_DEFAULT]
                    nop = mybir.InstNoOp(
                        name=nc.get_next_instruction_name(), ins=[], outs=[]
                    )
                    nop.engine = inst.engine
                    nop.sync_info = mybir.SyncInfo(on_wait=chunk, on_update=[])
                    nc.register_instruction(nop)
                    insts.insert(pos, nop)
                    pos += 1
                    i += 1
                i += 1


def build_conv_nc():
    """One-core program: x:(4,128,58,58) fp16 (pre-padded), wT:(128,9*256)
    fp16, bias2:(128,2) f32 -> out:(4,256,56,56) fp16."""
    nc = bass.Bass()
    x = nc.dram_tensor("x", [IMGS_PER_CORE, CIN, HP, WP], FP16, kind="ExternalInput")
    wt = nc.dram_tensor("wT", [CIN, 9 * COUT], FP16, kind="ExternalInput")
    bias2 = nc.dram_tensor("bias2", [128, 2], F32, kind="ExternalInput")
    out = nc.dram_tensor(
        "out", [IMGS_PER_CORE, COUT, H, W], FP16, kind="ExternalOutput"
    )

    with tile.TileContext(nc) as tc:
        with (
            tc.tile_pool(name="const", bufs=1) as const_pool,
            tc.tile_pool(name="xpad", bufs=1) as xpad_pool,
            tc.tile_pool(name="obuf", bufs=8) as obuf_pool,
            tc.tile_pool(name="psum", bufs=8, space="PSUM") as psum_pool,
        ):
            w_sb = const_pool.tile([CIN, 9 * COUT], FP16)
            b_sb = const_pool.tile([128, 2], F32)
            wz = const_pool.tile([CIN, NWZ], FP16)
            xpads = [
                xpad_pool.tile([CIN, HP, WP], FP16, tag=f"xpad{i}", name=f"xpad{i}")
                for i in range(4)
            ]

            # HAM warmup: memset-only dependency, so these issue right after
            # the framework preamble and keep the PE busy while input DMAs
            # stream. Results are never read.
            nc.vector.memset(wz[:], 0.0)
            for i in range(N_WARM):
                pw = psum_pool.tile([128, NWZ], F32, tag="ps", name=f"warm{i}")
                nc.tensor.matmul(pw[:], wz[:, 0:128], wz[:], start=True, stop=True)

            # Startup DMAs: rings split by ROLE, and per-ring FIFO used as a
            # rate limiter. Each dma_start costs ~0.6-1.2us of serial
            # descriptor-generation (DIRECT2D) on its ring's sequencer, all
            # rings share the 16 SDMA engines round-robin at packet
            # granularity, and a completion semaphore lags its data by
            # ~0.7-1us (write-receipt round trip). During the critical
            # 8-11.5us window only x rows 0-26 (384KB, sync ring) and the
            # weights+bias (584KB, scalar ring FIFO head) may touch SDMA;
            # all remaining input bulk is queued BEHIND the weights on the
            # scalar ring so FIFO order defers its execution until the tap
            # semaphores have fired. Outputs ride the sync ring (its NX is
            # idle after the two x gens, and IDENTITYs on the ACT NX never
            # get stuck behind a descriptor generation).
            # Layout wT[ci, (tap, chunk, co128)].
            nc.sync.dma_start(xpads[0][:, 0:10, :], x[0, :, 0:10, :])
            nc.sync.dma_start(xpads[0][:, 10:26, :], x[0, :, 10:26, :])
            nc.scalar.dma_start(w_sb[:, 0 : 4 * 128], wt[:, 0 : 4 * 128])
            nc.scalar.dma_start(w_sb[:, 4 * 128 : 10 * 128], wt[:, 4 * 128 : 10 * 128])
            nc.scalar.dma_start(w_sb[:, 10 * 128 :], wt[:, 10 * 128 :])
            nc.scalar.dma_start(b_sb[:], bias2[:])
            nc.scalar.dma_start(xpads[0][:, 26:42, :], x[0, :, 26:42, :])
            nc.scalar.dma_start(xpads[0][:, 42:58, :], x[0, :, 42:58, :])
            nc.scalar.dma_start(xpads[1][:], x[1])
            nc.scalar.dma_start(xpads[2][:], x[2])
            nc.scalar.dma_start(xpads[3][:], x[3])

            for img in range(IMGS_PER_CORE):
                xp = xpads[img]
                for t in range(N_ROW_TILES):
                    y0 = t * ROWS_PER_TILE
                    # The very last tile is split into row-halves so the
                    # final ACT+DMA+completion chain after the last matmul
                    # is half as deep.
                    last = img == IMGS_PER_CORE - 1 and t == N_ROW_TILES - 1
                    spans = (
                        [(y0, 4), (y0 + 4, 2), (y0 + 6, 2)]
                        if last
                        else [(y0, ROWS_PER_TILE)]
                    )
                    if img == 0 and t == 0:
                        # Consume taps 0/1 for BOTH chunks first: taps 2-4
                        # arrive on the scalar ring ~1us after taps 0-1, so
                        # this buys their completion an extra ~0.75us.
                        pss = [
                            psum_pool.tile(
                                [128, NTILE], F32, tag="ps", name=f"ps_0_0_{c}"
                            )
                            for c in range(2)
                        ]
                        t0_order = [(0, 0), (0, 1), (1, 0), (1, 1)] + [
                            (k, c) for c in range(2) for k in range(2, 9)
                        ]
                        for k, c in t0_order:
                            dr, dc = divmod(k, 3)
                            if dr == 0:
                                # Padded row 0 is all-zero: skip output row 0
                                # for the dr=0 taps. start=True still clears
                                # has_written for the WHOLE bank, so the
                                # untouched cols 0:56 are overwritten (not
                                # accumulated) by the first dr=1 tap.
                                rhs = xp[:, 1:ROWS_PER_TILE, dc : dc + W]
                                dst = pss[c][:, W:NTILE]
                            else:
                                rhs = xp[:, dr : dr + ROWS_PER_TILE, dc : dc + W]
                                dst = pss[c][:]
                            nc.tensor.matmul(
                                dst,
                                w_sb[:, (k * 2 + c) * 128 : (k * 2 + c) * 128 + 128],
                                rhs,
                                start=(k == 0),
                                stop=(k == 8),
                            )
                        for c in range(2):
                            ob = obuf_pool.tile(
                                [128, ROWS_PER_TILE, W], FP16, tag="ob",
                                name=f"ob_0_0_{c}",
                            )
                            nc.scalar.activation(
                                ob[:],
                                pss[c][:].rearrange("p (r w) -> p r w", w=W),
                                mybir.ActivationFunctionType.Identity,
                                bias=b_sb[:, c : c + 1],
                                scale=1.0,
                            )
                            nc.sync.dma_start(
                                out[0, c * 128 : (c + 1) * 128, 0:ROWS_PER_TILE, :],
                                ob[:],
                            )
                        continue
                    for c in range(2):  # Cout chunks of 128
                        for si, (r0, nr) in enumerate(spans):
                            nt = nr * W
                            ps = psum_pool.tile(
                                [128, nt], F32, tag="ps", name=f"ps_{img}_{r0}_{c}"
                            )
                            for k in range(9):
                                dr, dc = divmod(k, 3)
                                lhsT = w_sb[
                                    :, (k * 2 + c) * 128 : (k * 2 + c) * 128 + 128
                                ]
                                # Trim taps that would read the all-zero pad
                                # rows (padded row 0 for dr=0 in the first
                                # row-tile, padded row 57 for dr=2 in the
                                # last). start=True clears has_written for
                                # the whole bank, so untouched psum columns
                                # are overwritten by the first full tap.
                                if r0 == 0 and dr == 0:
                                    rhs = xp[:, 1 : r0 + nr, dc : dc + W]
                                    dst = ps[:, W:nt]
                                elif r0 + nr == H and dr == 2:
                                    rhs = xp[:, r0 + 2 : 57, dc : dc + W]
                                    dst = ps[:, 0 : nt - W]
                                else:
                                    rhs = xp[:, r0 + dr : r0 + dr + nr, dc : dc + W]
                                    dst = ps[:]
                                nc.tensor.matmul(
                                    dst, lhsT, rhs, start=(k == 0), stop=(k == 8)
                                )
                            ob = obuf_pool.tile(
                                [128, nr, W], FP16, tag="ob",
                                name=f"ob_{img}_{r0}_{c}",
                            )
                            if last and c == 0:
                                # Final tile: chunk-0 eviction on the (idle)
                                # DVE so the two chunks' PSUM->SBUF chains
                                # drain in parallel at the very end.
                                nc.vector.tensor_scalar_add(
                                    ob[:],
                                    ps[:].rearrange("p (r w) -> p r w", w=W),
                                    b_sb[:, 0:1],
                                )
                            else:
                                # out = Identity(psum*1.0 + bias[co]) on ScalarE
                                nc.scalar.activation(
                                    ob[:],
                                    ps[:].rearrange("p (r w) -> p r w", w=W),
                                    mybir.ActivationFunctionType.Identity,
                                    bias=b_sb[:, c : c + 1],
                                    scale=1.0,
                                )
                            nc.sync.dma_start(
                                out[img, c * 128 : (c + 1) * 128, r0 : r0 + nr, :],
                                ob[:],
                            )

    _cap_sync_waits(nc)
    nc.finalize()
    return nc


_NC_CACHE = {}


def _get_nc():
    if "nc" not in _NC_CACHE:
        _NC_CACHE["nc"] = build_conv_nc()
    return _NC_CACHE["nc"]


def _prep_in_maps(x, weight, bias):
    x = np.asarray(x, dtype=np.float32)
    n = x.shape[0]
    # pad to 58x58 and cast fp16 once, full batch
    xp = np.zeros((n, CIN, HP, WP), dtype=np.float16)
    xp[:, :, 1 : H + 1, 1 : W + 1] = x
    # weight (256,128,3,3) -> wT[ci, (tap, chunk, co128)] fp16
    wT = (
        np.transpose(np.asarray(weight, dtype=np.float32), (1, 2, 3, 0))
        .reshape(CIN, 9, 2, 128)
        .reshape(CIN, 9 * COUT)
        .astype(np.float16)
    )
    wT = np.ascontiguousarray(wT)
    bias2 = np.ascontiguousarray(
        np.asarray(bias, dtype=np.float32).reshape(2, 128).T
    )
    per_core = n // N_CORES
    return [
        {
            "x": np.ascontiguousarray(xp[i * per_core : (i + 1) * per_core]),
            "wT": wT,
            "bias2": bias2,
        }
        for i in range(N_CORES)
    ]


def run(x, weight, bias, trace=False):
    """Run the conv on 8 cores; returns (out, BassKernelResults)."""
    nc = _get_nc()
    in_maps = _prep_in_maps(x, weight, bias)
    res = run_bass_kernel_spmd(
        nc, in_maps, core_ids=list(range(N_CORES)), trace=trace
    )
    out = np.concatenate([r["out"] for r in res.results], axis=0).astype(np.float32)
    return out, res


def kernel(x, weight, bias):
    out, _ = run(x, weight, bias, trace=False)
    return out



# revision 16
# speedup vs baseline: 1.0179x; 1.0025x over previous
"""Trainium2 Bass kernel for nn_Conv_6511170421767.

3x3 conv, stride 1, pad 1 on x:(32,128,56,56) with weight:(256,128,3,3),
bias:(256,) -> out:(32,256,56,56), fp32 in/out.

Strategy (data-parallel, 4 images per core on 8 cores):
- Cin=128 is exactly the PE contraction/partition dim. The conv becomes,
  per (output-row-block, Cout-chunk), an accumulation of 9 matmuls (one per
  kernel tap): out[co, pix] += W[dr,dc][ci,co].T @ xpad[ci, shifted pix].
- The host pre-pads x to (4,128,58,58) fp16 and pre-transposes/casts the
  weights, so input DMAs land directly in the padded SBUF plane and the
  on-chip DVE does ZERO work (v1 spent ~26us of DVE on casts/borders).
- Matmul operands are fp16 (1 PE cycle/row - vs 4 for plain fp32 - with a
  10-bit mantissa; operand ranges here sit safely inside fp16's range).
  Accumulation is fp32 in PSUM. Measured rel err vs fp32 reference: ~3e-4.
- PSUM tile [128, 448] = one bank; 9 taps accumulate in-bank, then the
  scalar engine adds bias (Identity activation w/ per-partition bias AP)
  while copying PSUM->SBUF as fp16; the host upcasts fp16->fp32. The very
  last tile is split into row-halves to shorten the final ACT+DMA+
  completion chain. Output DMAs ride the sync ring (it keeps pace; ~0.6us
  descriptor cost each, paced ~1.7us apart by compute).
- Cold-start: the PE HAM clock gate runs matmuls at 1.2GHz until ~3.4us of
  CONTINUOUS PE-busy (any idle gap restarts the window). Dependency-free
  warmup matmuls (zero fp16 operands from a DVE memset, never read back)
  start right after the ~7.5us fixed framework preamble and bridge into
  the first data-gated matmul, so the HAM flips to 2.4GHz ~1.8us into the
  real stream. Startup transfers are deadline-ordered across BOTH HWDGE
  rings: sync = [x rows 0-9, taps 5-8, x rows 10-25, bias, x rest, img1,
  img2], scalar = [taps 0-1, taps 2-4] (kept free of bulk so the tap
  completion semaphores land before their first LDWEIGHTS; a transfer's
  completion lags its issue by ~2-2.5us). Each DMA's completion semaphore
  is per-transfer, hence the deadline-sized pieces.

The external neuronxcc walrus in this container enforces small per-
instruction sync-wait limits (TRN2 HW allows 1 per instruction);
_cap_sync_waits() splits excess waits onto InstNoOp instructions inserted
just before the offender on the same engine.
"""

import sys

sys.path.insert(0, "/opt/trn_rl_repo")

import numpy as np

import concourse.bass as bass
import concourse.mybir as mybir
import concourse.tile as tile
from concourse.bass_utils import run_bass_kernel_spmd

F32 = mybir.dt.float32
FP16 = mybir.dt.float16

N_CORES = 8
IMGS_PER_CORE = 4
CIN = 128
COUT = 256
H = W = 56
HP = WP = 58  # padded plane
ROWS_PER_TILE = 8  # 8 output rows -> N = 448 <= 512 (one PSUM bank)
N_ROW_TILES = H // ROWS_PER_TILE  # 7
NTILE = ROWS_PER_TILE * W  # 448
N_WARM = 13  # dependency-free HAM-warmup matmuls (N=224 each, ~190ns cold)
NWZ = 224  # warmup matmul free dim

# Per-instruction sync-wait budget for the external walrus: TRN2 hardware
# allows at most 1 sync wait per instruction.
_WAIT_LIMITS_DEFAULT = 1
_WAIT_LIMITS = {}


def _cap_sync_waits(nc):
    """Split sync waits exceeding per-instruction limits onto same-engine
    InstNoOp instructions inserted immediately before the offender."""
    for fn in nc.m.functions:
        for bb in fn.blocks:
            i = 0
            insts = bb.instructions
            while i < len(insts):
                inst = insts[i]
                si = getattr(inst, "sync_info", None)
                if si is None or not si.on_wait:
                    i += 1
                    continue
                limit = _WAIT_LIMITS.get(type(inst).__name__, _WAIT_LIMITS_DEFAULT)
                waits = list(si.on_wait)
                if len(waits) <= limit:
                    i += 1
                    continue
                keep = waits[:limit]
                excess = waits[limit:]
                inst.sync_info = mybir.SyncInfo(
                    on_wait=keep, on_update=list(si.on_update)
                )
                pos = i
                for j in range(0, len(excess), _WAIT_LIMITS_DEFAULT):
                    chunk = excess[j : j + _WAIT_LIMITS_DEFAULT]
                    nop = mybir.InstNoOp(
                        name=nc.get_next_instruction_name(), ins=[], outs=[]
                    )
                    nop.engine = inst.engine
                    nop.sync_info = mybir.SyncInfo(on_wait=chunk, on_update=[])
                    nc.register_instruction(nop)
                    insts.insert(pos, nop)
                    pos += 1
                    i += 1
                i += 1


def build_conv_nc():
    """One-core program: x:(4,128,58,58) fp16 (pre-padded), wT:(128,9*256)
    fp16, bias2:(128,2) f32 -> out:(4,256,56,56) fp16."""
    nc = bass.Bass()
    x = nc.dram_tensor("x", [IMGS_PER_CORE, CIN, HP, WP], FP16, kind="ExternalInput")
    wt = nc.dram_tensor("wT", [CIN, 9 * COUT], FP16, kind="ExternalInput")
    bias2 = nc.dram_tensor("bias2", [128, 2], F32, kind="ExternalInput")
    out = nc.dram_tensor(
        "out", [IMGS_PER_CORE, COUT, H, W], FP16, kind="ExternalOutput"
    )

    with tile.TileContext(nc) as tc:
        with (
            tc.tile_pool(name="const", bufs=1) as const_pool,
            tc.tile_pool(name="xpad", bufs=1) as xpad_pool,
            tc.tile_pool(name="obuf", bufs=8) as obuf_pool,
            tc.tile_pool(name="psum", bufs=8, space="PSUM") as psum_pool,
        ):
            w_sb = const_pool.tile([CIN, 9 * COUT], FP16)
            b_sb = const_pool.tile([128, 2], F32)
            wz = const_pool.tile([CIN, NWZ], FP16)
            xpads = [
                xpad_pool.tile([CIN, HP, WP], FP16, tag=f"xpad{i}", name=f"xpad{i}")
                for i in range(4)
            ]

            # HAM warmup: memset-only dependency, so these issue right after
            # the framework preamble and keep the PE busy while input DMAs
            # stream. Results are never read.
            nc.vector.memset(wz[:], 0.0)
            for i in range(N_WARM):
                pw = psum_pool.tile([128, NWZ], F32, tag="ps", name=f"warm{i}")
                nc.tensor.matmul(pw[:], wz[:, 0:128], wz[:], start=True, stop=True)

            # Startup DMAs: measured law — WITHIN one DMA chain (sync HWDGE
            # ring / scalar HWDGE ring / gpsimd SWDGE queue) consecutive
            # transfers serialize at ~1.4-1.6us apiece: the next transfer's
            # data does not move until the previous transfer's completion
            # semaphore (a write-receipt round trip, ~0.6-1us after last
            # byte) has retired. Tap weights are consumed every ~0.7us by
            # tile0, so they are LADDERED over two chains (scalar: taps
            # 0,1 / 4,5 / 8; gpsimd: taps 2,3 / 6,7) matching the k-major
            # consumption order of tile0; image bulk flows down sync (rows
            # 0-26 of img0) and gpsimd (rest), never ahead of weights on
            # their chain. Layout wT[ci, (tap, chunk, co128)]: tap k =
            # cols k*256:(k+1)*256.
            nc.sync.dma_start(xpads[0][:, 0:10, :], x[0, :, 0:10, :])
            nc.sync.dma_start(xpads[0][:, 10:26, :], x[0, :, 10:26, :])
            nc.scalar.dma_start(w_sb[:, 0:512], wt[:, 0:512])
            nc.scalar.dma_start(w_sb[:, 1024:1536], wt[:, 1024:1536])
            nc.scalar.dma_start(w_sb[:, 2048:2304], wt[:, 2048:2304])
            nc.scalar.dma_start(b_sb[:], bias2[:])
            nc.gpsimd.dma_start(w_sb[:, 512:1024], wt[:, 512:1024])
            nc.gpsimd.dma_start(w_sb[:, 1536:2048], wt[:, 1536:2048])
            nc.gpsimd.dma_start(xpads[0][:, 26:42, :], x[0, :, 26:42, :])
            nc.gpsimd.dma_start(xpads[0][:, 42:58, :], x[0, :, 42:58, :])
            nc.gpsimd.dma_start(xpads[1][:], x[1])
            nc.gpsimd.dma_start(xpads[2][:], x[2])
            nc.gpsimd.dma_start(xpads[3][:], x[3])

            for img in range(IMGS_PER_CORE):
                xp = xpads[img]
                for t in range(N_ROW_TILES):
                    y0 = t * ROWS_PER_TILE
                    # The last tile is NOT split: within one DMA chain,
                    # consecutive transfers serialize at ~1.5us (receipt
                    # chaining), so several small final DMAs finish LATER
                    # than one 8-row DMA per chunk on two separate chains.
                    last = img == IMGS_PER_CORE - 1 and t == N_ROW_TILES - 1
                    spans = [(y0, ROWS_PER_TILE)]
                    if img == 0 and t == 0:
                        # Consume taps 0/1 for BOTH chunks first: taps 2-4
                        # arrive on the scalar ring ~1us after taps 0-1, so
                        # this buys their completion an extra ~0.75us.
                        pss = [
                            psum_pool.tile(
                                [128, NTILE], F32, tag="ps", name=f"ps_0_0_{c}"
                            )
                            for c in range(2)
                        ]
                        # k-major: first-use of tap k advances one rung
                        # (~0.7us) at a time, matching the two weight-chain
                        # semaphore ladders.
                        t0_order = [(0, 0), (0, 1), (1, 0), (1, 1)] + [
                            (k, c) for k in range(2, 9) for c in range(2)
                        ]
                        for k, c in t0_order:
                            dr, dc = divmod(k, 3)
                            if dr == 0:
                                # Padded row 0 is all-zero: skip output row 0
                                # for the dr=0 taps. start=True still clears
                                # has_written for the WHOLE bank, so the
                                # untouched cols 0:56 are overwritten (not
                                # accumulated) by the first dr=1 tap.
                                rhs = xp[:, 1:ROWS_PER_TILE, dc : dc + W]
                                dst = pss[c][:, W:NTILE]
                            else:
                                rhs = xp[:, dr : dr + ROWS_PER_TILE, dc : dc + W]
                                dst = pss[c][:]
                            nc.tensor.matmul(
                                dst,
                                w_sb[:, (k * 2 + c) * 128 : (k * 2 + c) * 128 + 128],
                                rhs,
                                start=(k == 0),
                                stop=(k == 8),
                            )
                        for c in range(2):
                            ob = obuf_pool.tile(
                                [128, ROWS_PER_TILE, W], FP16, tag="ob",
                                name=f"ob_0_0_{c}",
                            )
                            nc.scalar.activation(
                                ob[:],
                                pss[c][:].rearrange("p (r w) -> p r w", w=W),
                                mybir.ActivationFunctionType.Identity,
                                bias=b_sb[:, c : c + 1],
                                scale=1.0,
                            )
                            oring = nc.sync if c == 0 else nc.scalar
                            oring.dma_start(
                                out[0, c * 128 : (c + 1) * 128, 0:ROWS_PER_TILE, :],
                                ob[:],
                            )
                        continue
                    for c in range(2):  # Cout chunks of 128
                        for si, (r0, nr) in enumerate(spans):
                            nt = nr * W
                            ps = psum_pool.tile(
                                [128, nt], F32, tag="ps", name=f"ps_{img}_{r0}_{c}"
                            )
                            for k in range(9):
                                dr, dc = divmod(k, 3)
                                lhsT = w_sb[
                                    :, (k * 2 + c) * 128 : (k * 2 + c) * 128 + 128
                                ]
                                # Trim taps that would read the all-zero pad
                                # rows (padded row 0 for dr=0 in the first
                                # row-tile, padded row 57 for dr=2 in the
                                # last). start=True clears has_written for
                                # the whole bank, so untouched psum columns
                                # are overwritten by the first full tap.
                                if r0 == 0 and dr == 0:
                                    rhs = xp[:, 1 : r0 + nr, dc : dc + W]
                                    dst = ps[:, W:nt]
                                elif r0 + nr == H and dr == 2:
                                    rhs = xp[:, r0 + 2 : 57, dc : dc + W]
                                    dst = ps[:, 0 : nt - W]
                                else:
                                    rhs = xp[:, r0 + dr : r0 + dr + nr, dc : dc + W]
                                    dst = ps[:]
                                nc.tensor.matmul(
                                    dst, lhsT, rhs, start=(k == 0), stop=(k == 8)
                                )
                            ob = obuf_pool.tile(
                                [128, nr, W], FP16, tag="ob",
                                name=f"ob_{img}_{r0}_{c}",
                            )
                            if last and c == 0:
                                # Final tile: chunk-0 eviction on the (idle)
                                # DVE so the two chunks' PSUM->SBUF->HBM
                                # chains drain fully in parallel at the end
                                # (c0: DVE + sync ring, c1: ScalarE + scalar
                                # ring).
                                nc.vector.tensor_scalar_add(
                                    ob[:],
                                    ps[:].rearrange("p (r w) -> p r w", w=W),
                                    b_sb[:, 0:1],
                                )
                            else:
                                # out = Identity(psum*1.0 + bias[co]) on ScalarE
                                nc.scalar.activation(
                                    ob[:],
                                    ps[:].rearrange("p (r w) -> p r w", w=W),
                                    mybir.ActivationFunctionType.Identity,
                                    bias=b_sb[:, c : c + 1],
                                    scale=1.0,
                                )
                            # c0 -> sync chain, c1 -> scalar chain: halves
                            # each chain's load and lets the two final
                            # transfers complete concurrently.
                            oring = nc.sync if c == 0 else nc.scalar
                            oring.dma_start(
                                out[img, c * 128 : (c + 1) * 128, r0 : r0 + nr, :],
                                ob[:],
                            )

    _cap_sync_waits(nc)
    nc.finalize()
    return nc


_NC_CACHE = {}


def _get_nc():
    if "nc" not in _NC_CACHE:
        _NC_CACHE["nc"] = build_conv_nc()
    return _NC_CACHE["nc"]


def _prep_in_maps(x, weight, bias):
    x = np.asarray(x, dtype=np.float32)
    n = x.shape[0]
    # pad to 58x58 and cast fp16 once, full batch
    xp = np.zeros((n, CIN, HP, WP), dtype=np.float16)
    xp[:, :, 1 : H + 1, 1 : W + 1] = x
    # weight (256,128,3,3) -> wT[ci, (tap, chunk, co128)] fp16
    wT = (
        np.transpose(np.asarray(weight, dtype=np.float32), (1, 2, 3, 0))
        .reshape(CIN, 9, 2, 128)
        .reshape(CIN, 9 * COUT)
        .astype(np.float16)
    )
    wT = np.ascontiguousarray(wT)
    bias2 = np.ascontiguousarray(
        np.asarray(bias, dtype=np.float32).reshape(2, 128).T
    )
    per_core = n // N_CORES
    return [
        {
            "x": np.ascontiguousarray(xp[i * per_core : (i + 1) * per_core]),
            "wT": wT,
            "bias2": bias2,
        }
        for i in range(N_CORES)
    ]


def run(x, weight, bias, trace=False):
    """Run the conv on 8 cores; returns (out, BassKernelResults)."""
    nc = _get_nc()
    in_maps = _prep_in_maps(x, weight, bias)
    res = run_bass_kernel_spmd(
        nc, in_maps, core_ids=list(range(N_CORES)), trace=trace
    )
    out = np.concatenate([r["out"] for r in res.results], axis=0).astype(np.float32)
    return out, res


def kernel(x, weight, bias):
    out, _ = run(x, weight, bias, trace=False)
    return out

